# revision 1
# baseline (speedup 1.0000x reference)
"""GCN layer kernel for Trainium2, 8 NeuronCores.

Math (identical to reference):
    deg = bincount(row);  d = 1/sqrt(deg)
    h   = x @ W.T + b
    out = d * segment_sum(d[col] * h[col], row) + d^2 * h

Rewritten as aggregate-then-transform (linear map commutes with segment sum):
    y[j]   = d_j * x_j                                   (built on device, launch 1)
    U[r]   = sum_{edges (r,c)} y[c] + y[r]               (self term = extra edge slot)
    cc[r]  = sum_{edges (r,c)} d_c + d_r
    out[r] = d_r * (U[r] @ W.T + cc[r] * b)

Launch 2 (destinations sharded across the 8 cores, SPMD — identical program,
per-core data):
  * edges sorted by (dest block of 128, source chunk of 25088); gathered in
    bulk with gpsimd.dma_gather (512B y rows, full DMA rate).  Gathered edge
    i lands at SBUF partition i%128, tile i//128.
  * per 128-edge tile, a 0/1 selection matrix S[e, dest_local] is built with
    one tensor_scalar is_equal against an iota row (split between DVE and
    GpSimd), and one PE matmul accumulates S^T @ G into the block's PSUM U.
  * cc comes from a second, dest-major layout of per-edge degrees:
    rsqrt then a unit-stride tensor_reduce per block (cheap).
  * per block: PE transpose of U, 128x128 matmul with W^T, then scale/bias.
Slot padding uses source row 0 with dest_local = -1 (S column is all zero),
so padded gathers are harmless; per-(block,chunk) tile counts are the max
over cores, keeping shapes static across the SPMD program.
"""

import numpy as np
import sys

sys.path.insert(0, "/opt/trn_rl_repo")

import concourse.bacc as bacc
import concourse.tile as tile
from concourse import mybir
from concourse.bass_utils import run_bass_kernel_spmd
from concourse.masks import make_identity

NCORES = 8
P = 128
CHUNK = 25088  # dma_gather idx is int16: source chunks must stay < 32768 rows
SB = 4  # dest blocks per superblock (gather granularity)
SLAB_BUFS = 2
S_GP_8THS = 0  # selection-matrix builds on DVE only (GpSimd time is needed for gather descriptor generation)
F32 = mybir.dt.float32
I16 = mybir.dt.int16
BF16 = mybir.dt.bfloat16

_cache = {}
LAST = {}  # populated on each kernel() call (for profiling in test.py)


def _build_launch1(npc_pad, din):
    """Per-core: y_sh[i] = d_i * x_i for the core's node shard."""
    ntiles = npc_pad // P
    nc = bacc.Bacc(
        "TRN2",
        target_bir_lowering=False,
        debug=False,
        enable_asserts=False,
        num_devices=NCORES,
    )
    x_sh = nc.dram_tensor("x_sh", [npc_pad, din], F32, kind="ExternalInput").ap()
    deg_sh = nc.dram_tensor("deg_sh", [P, ntiles], F32, kind="ExternalInput").ap()
    y_sh = nc.dram_tensor("y_sh", [npc_pad, 2 * din], BF16, kind="ExternalOutput").ap()

    x_v = x_sh.rearrange("(t p) f -> p t f", p=P)
    y_v = y_sh.rearrange("(t p) f -> p t f", p=P)  # f = 2*din (bf16 hi|lo)
    CT = 14  # tiles per pipeline chunk
    with tile.TileContext(nc) as tc:
        with (
            tc.tile_pool(name="const", bufs=1) as cpool,
            tc.tile_pool(name="sb", bufs=3) as pool,
        ):
            deg_sb = cpool.tile([P, ntiles], dtype=F32)
            nc.sync.dma_start(out=deg_sb[:], in_=deg_sh[:, :])
            sq = cpool.tile([P, ntiles], dtype=F32)
            nc.scalar.activation(sq[:], deg_sb[:], mybir.ActivationFunctionType.Sqrt)
            d_all = cpool.tile([P, ntiles], dtype=F32)
            nc.vector.reciprocal(d_all[:], sq[:])
            for c0 in range(0, ntiles, CT):
                nt = min(CT, ntiles - c0)
                xt = pool.tile([P, CT, din], dtype=F32, tag="xt")
                nc.sync.dma_start(out=xt[:, 0:nt, :], in_=x_v[:, c0 : c0 + nt, :])
                yt = pool.tile([P, CT, din], dtype=F32, tag="yt")
                hf = pool.tile([P, CT, din], dtype=F32, tag="hf")
                yp = pool.tile([P, CT, 2 * din], dtype=BF16, tag="yp")
                for k in range(nt):
                    nc.vector.tensor_scalar(
                        out=yt[:, k, :],
                        in0=xt[:, k, :],
                        scalar1=d_all[:, c0 + k : c0 + k + 1],
                        scalar2=None,
                        op0=mybir.AluOpType.mult,
                    )
                    # hi = bf16(y); lo = bf16(y - f32(hi))
                    nc.vector.tensor_copy(yp[:, k, 0:din], yt[:, k, :])
                    nc.scalar.activation(
                        hf[:, k, :], yp[:, k, 0:din], mybir.ActivationFunctionType.Copy
                    )
                    nc.vector.tensor_tensor(
                        out=yp[:, k, din : 2 * din],
                        in0=yt[:, k, :],
                        in1=hf[:, k, :],
                        op=mybir.AluOpType.subtract,
                    )
                nc.sync.dma_start(out=y_v[:, c0 : c0 + nt, :], in_=yp[:, 0:nt, :])
    nc.compile()
    return nc


def _build_launch2(meta):
    """Gather + selection-matmul segment sum + per-block W matmul."""
    din = meta["din"]
    dout = meta["dout"]
    nchunk = meta["nchunk"]
    n_y = meta["n_y"]  # padded y rows (nchunk * CHUNK)
    nblk = meta["nblk"]
    ttot = meta["ttot"]  # total 128-edge tiles
    ktot = meta["ktot"]  # total dest-major slots for cc
    koff = meta["koff"]  # [nblk+1]
    sblocks = meta["sblocks"]  # list of lists of block ids
    sb_tiles = meta["sb_tiles"]  # per sb: total tiles
    sb_calls = meta["sb_calls"]  # per sb: list of (chunk, tile_off_in_sb, ntiles)
    blk_runs = meta["blk_runs"]  # per block: list of (tile_off_in_sb, ntiles)
    sb_of_blk = meta["sb_of_blk"]
    tile_base = meta["tile_base"]  # per sb: global tile offset (for dl indexing)
    win_lo = meta["win_lo"]
    win_w = meta["win_w"]
    koff = meta["koff"]

    nc = bacc.Bacc(
        "TRN2",
        target_bir_lowering=False,
        debug=False,
        enable_asserts=False,
        num_devices=NCORES,
    )
    y_t = nc.dram_tensor("y_t", [n_y, 2 * din], BF16, kind="ExternalInput").ap()
    ys_t = nc.dram_tensor("ys_t", [nblk * P, 2 * din], BF16, kind="ExternalInput").ap()
    idx_t = nc.dram_tensor("idx_t", [P, ttot * 8], I16, kind="ExternalInput").ap()
    dl_t = nc.dram_tensor("dl_t", [P, ttot], F32, kind="ExternalInput").ap()
    degE_t = nc.dram_tensor("degE_t", [P, ktot], F32, kind="ExternalInput").ap()
    wt_t = nc.dram_tensor("wt_t", [din, dout], F32, kind="ExternalInput").ap()
    brep_t = nc.dram_tensor("brep_t", [P, dout], F32, kind="ExternalInput").ap()
    out_t = nc.dram_tensor("out_t", [nblk * P, dout], F32, kind="ExternalOutput").ap()

    max_sb_tiles = max(sb_tiles)

    with tile.TileContext(nc) as tc:
        with (
            tc.tile_pool(name="const", bufs=1) as cpool,
            tc.tile_pool(name="slab", bufs=SLAB_BUFS) as gpool,
            tc.tile_pool(name="sel", bufs=6) as selpool,
            tc.tile_pool(name="work", bufs=3) as wpool,
            tc.tile_pool(name="small", bufs=4) as spool,
            tc.tile_pool(name="psum", bufs=2, space="PSUM") as ppool,
        ):
            ident = cpool.tile([P, P], dtype=F32)
            make_identity(nc, ident[:])
            ident_bf = cpool.tile([P, P], dtype=BF16)
            nc.vector.tensor_copy(ident_bf[:], ident[:])
            iota_i = cpool.tile([P, P], dtype=mybir.dt.int32)
            nc.gpsimd.iota(iota_i[:], pattern=[[1, P]], base=0, channel_multiplier=0)
            iota_f = cpool.tile([P, P], dtype=BF16)
            nc.vector.tensor_copy(iota_f[:], iota_i[:])
            wt_sb = cpool.tile([din, dout], dtype=F32)
            nc.sync.dma_start(out=wt_sb[:], in_=wt_t[:, :])
            brep_sb = cpool.tile([P, dout], dtype=F32)
            nc.sync.dma_start(out=brep_sb[:], in_=brep_t[:, :])
            degE_sb = cpool.tile([P, ktot], dtype=F32)
            nc.sync.dma_start(out=degE_sb[:], in_=degE_t[:, :])
            sqE = cpool.tile([P, ktot], dtype=F32)
            nc.scalar.activation(sqE[:], degE_sb[:], mybir.ActivationFunctionType.Sqrt)
            sE = cpool.tile([P, ktot], dtype=F32)
            nc.vector.reciprocal(sE[:], sqE[:])

            ys_v = ys_t.rearrange("(t p) f -> p t f", p=P)
            out_v = out_t.rearrange("(t p) f -> p t f", p=P)
            for sbi, blks in enumerate(sblocks):
                nt_sb = sb_tiles[sbi]
                tb = tile_base[sbi]
                nb = len(blks)
                idx_sb = wpool.tile([P, max_sb_tiles * 8], dtype=I16, tag="idx")
                nc.sync.dma_start(
                    out=idx_sb[:, 0 : nt_sb * 8],
                    in_=idx_t[:, tb * 8 : (tb + nt_sb) * 8],
                )
                dl_sb = wpool.tile([P, max_sb_tiles], dtype=F32, tag="dl")
                nc.sync.dma_start(out=dl_sb[:, 0:nt_sb], in_=dl_t[:, tb : tb + nt_sb])
                ys_sb = wpool.tile([P, SB, 2 * din], dtype=BF16, tag="ys")
                nc.sync.dma_start(
                    out=ys_sb[:, 0:nb, :], in_=ys_v[:, blks[0] : blks[0] + nb, :]
                )
                slab = gpool.tile([P, max_sb_tiles, 2 * din], dtype=BF16, tag="slab")
                for (c, toff, nt) in sb_calls[sbi]:
                    ni = nt * P
                    nc.gpsimd.dma_gather(
                        out_ap=slab[:, toff : toff + nt, :],
                        in_ap=y_t[c * CHUNK : (c + 1) * CHUNK, :],
                        idxs_ap=idx_sb[:, toff * 8 : (toff + nt) * 8],
                        num_idxs=ni,
                        num_idxs_reg=ni,
                        elem_size=2 * din,
                        single_packet=False,
                    )
                osb_sb = wpool.tile([P, SB, dout], dtype=F32, tag="osb")
                for j, b in enumerate(blks):
                    ups = ppool.tile([P, 2 * din], dtype=F32, space="PSUM", tag="ups")
                    runs = blk_runs[b]
                    ntb = sum(r[1] for r in runs)
                    # self term first (full 128 rows -> clears the whole PSUM tile)
                    nc.tensor.matmul(
                        out=ups[:],
                        lhsT=ident_bf[:],
                        rhs=ys_sb[:, j, :],
                        start=True,
                        stop=(ntb == 0),
                    )
                    ti = 0
                    for (toff, nt) in runs:
                        for k in range(nt):
                            t_sb = toff + k
                            t_g = tb + t_sb
                            lo = int(win_lo[t_g])
                            w = int(win_w[t_g])
                            st = selpool.tile([P, P], dtype=BF16, tag="st")
                            nc.vector.tensor_scalar(
                                out=st[:, 0:w],
                                in0=iota_f[:, 0:w],
                                scalar1=dl_sb[:, t_sb : t_sb + 1],
                                scalar2=None,
                                op0=mybir.AluOpType.is_equal,
                            )
                            nc.tensor.matmul(
                                out=ups[lo : lo + w, :],
                                lhsT=st[:, 0:w],
                                rhs=slab[:, t_sb, :],
                                start=False,
                                stop=(ti == ntb - 1),
                            )
                            ti += 1
                    # cc and d for this block (dest-major degree layout)
                    ko, k1 = int(koff[b]), int(koff[b + 1])
                    cc = spool.tile([P, 1], dtype=F32, tag="cc")
                    nc.vector.tensor_reduce(
                        out=cc[:],
                        in_=sE[:, ko:k1],
                        axis=mybir.AxisListType.X,
                        op=mybir.AluOpType.add,
                    )
                    # U = hi-part + lo-part
                    upsb = wpool.tile([P, 2 * din], dtype=F32, tag="upsb")
                    nc.scalar.activation(
                        upsb[:], ups[:], mybir.ActivationFunctionType.Copy
                    )
                    usb = wpool.tile([P, din], dtype=F32, tag="usb")
                    nc.vector.tensor_tensor(
                        out=usb[:],
                        in0=upsb[:, 0:din],
                        in1=upsb[:, din : 2 * din],
                        op=mybir.AluOpType.add,
                    )
                    utp = ppool.tile([P, P], dtype=F32, space="PSUM", tag="utp")
                    nc.tensor.transpose(out=utp[:], in_=usb[:], identity=ident[:])
                    uts = wpool.tile([din, P], dtype=F32, tag="uts")
                    nc.scalar.activation(
                        uts[:], utp[:], mybir.ActivationFunctionType.Copy
                    )
                    o2 = ppool.tile([P, dout], dtype=F32, space="PSUM", tag="o2")
                    nc.tensor.matmul(
                        out=o2[:], lhsT=uts[:], rhs=wt_sb[:], start=True, stop=True
                    )
                    # out = d * (U @ W.T) + (cc * d) * b    (d = sE slot 0)
                    cd = spool.tile([P, 1], dtype=F32, tag="cd")
                    nc.vector.tensor_tensor(
                        out=cd[:],
                        in0=cc[:],
                        in1=sE[:, ko : ko + 1],
                        op=mybir.AluOpType.mult,
                    )
                    t1 = wpool.tile([P, dout], dtype=F32, tag="t1")
                    nc.scalar.activation(
                        t1[:],
                        brep_sb[:],
                        mybir.ActivationFunctionType.Copy,
                        scale=cd[:, 0:1],
                    )
                    t2 = wpool.tile([P, dout], dtype=F32, tag="t2")
                    nc.scalar.activation(
                        t2[:],
                        o2[:],
                        mybir.ActivationFunctionType.Copy,
                        scale=sE[:, ko : ko + 1],
                    )
                    nc.vector.tensor_tensor(
                        out=osb_sb[:, j, :],
                        in0=t2[:],
                        in1=t1[:],
                        op=mybir.AluOpType.add,
                    )
                nc.sync.dma_start(
                    out=out_v[:, blks[0] : blks[0] + nb, :], in_=osb_sb[:, 0:nb, :]
                )
    nc.compile()
    return nc


def _prep(x, edge_index, W, b):
    N, din = x.shape
    dout = W.shape[0]
    npc = N // NCORES
    nblk = (npc + P - 1) // P
    npc_pad = nblk * P
    nchunk = (N + CHUNK - 1) // CHUNK
    n_y = nchunk * CHUNK

    row = np.asarray(edge_index[0], dtype=np.int64)
    col = np.asarray(edge_index[1], dtype=np.int64)
    deg = np.bincount(row, minlength=N)  # int
    order_e = np.argsort(row, kind="stable")
    row_s = row[order_e]
    col_s = col[order_e]
    rowstart = np.zeros(N + 1, dtype=np.int64)
    np.cumsum(deg, out=rowstart[1:])

    # ---- per-core edge lists (dest-sharded), with self edges appended -------
    # per core arrays: dest_local(0..npc_pad), col (global), sorted by
    # (block, chunk) with CSR order preserved inside.
    core_dl = []
    core_col = []
    counts = np.zeros((NCORES, nblk, nchunk), dtype=np.int64)
    for m in range(NCORES):
        lo, hi = rowstart[m * npc], rowstart[(m + 1) * npc]
        dl = row_s[lo:hi] - m * npc
        cl = col_s[lo:hi]
        # (self-loop term handled via the per-core yself input, not gathered)
        blk = dl >> 7
        ch = cl // CHUNK
        o = np.lexsort((ch, blk))
        dl, cl, blk, ch = dl[o], cl[o], blk[o], ch[o]
        core_dl.append(dl)
        core_col.append(cl)
        np.add.at(counts[m], (blk, ch), 1)

    tcnt = (np.max(counts, axis=0) + P - 1) // P  # [nblk, nchunk] tiles
    # ---- static tile schedule ----------------------------------------------
    sblocks = [list(range(s, min(s + SB, nblk))) for s in range(0, nblk, SB)]
    sb_calls = []
    blk_runs = [None] * nblk
    sb_of_blk = [0] * nblk
    sb_tiles = []
    tile_base = []
    tpos = {}  # (b, c) -> global tile offset
    gt = 0
    for sbi, blks in enumerate(sblocks):
        tile_base.append(gt)
        calls = []
        toff = 0
        for c in range(nchunk):
            nt = int(sum(tcnt[b, c] for b in blks))
            if nt:
                calls.append((c, toff, nt))
            for b in blks:
                if tcnt[b, c]:
                    tpos[(b, c)] = gt + toff
                    toff += int(tcnt[b, c])
        sb_calls.append(calls)
        for b in blks:
            sb_of_blk[b] = sbi
            blk_runs[b] = [
                (tpos[(b, c)] - gt, int(tcnt[b, c]))
                for c in range(nchunk)
                if tcnt[b, c]
            ]
        sb_tiles.append(toff)
        gt += toff
    ttot = gt

    # ---- per-core slot data -------------------------------------------------
    import ml_dtypes
    bf16 = np.dtype(ml_dtypes.bfloat16)
    idx_all = np.zeros((NCORES, P, ttot * 8), dtype=np.int16)
    dlf_all = np.full((NCORES, ttot, P), -1.0, dtype=np.float32)
    for m in range(NCORES):
        dl, cl = core_dl[m], core_col[m]
        blk = dl >> 7
        ch = cl // CHUNK
        # slot position of each edge: tiles of its (blk,ch) group, CSR order
        gkey = blk * nchunk + ch
        gcnt = np.bincount(gkey, minlength=nblk * nchunk).reshape(nblk, nchunk)
        starts128 = np.zeros((nblk, nchunk), dtype=np.int64)
        for b in range(nblk):
            for c in range(nchunk):
                if tcnt[b, c]:
                    starts128[b, c] = tpos[(b, c)] * P
        # position within group
        grp_start = np.zeros(nblk * nchunk + 1, dtype=np.int64)
        np.cumsum(gcnt.ravel(), out=grp_start[1:])
        within = np.arange(len(dl), dtype=np.int64) - grp_start[gkey]
        slot = starts128[blk, ch] + within
        tno = slot >> 7
        pno = slot & 127
        lcol = (cl - ch * CHUNK).astype(np.int16)
        # wrapped idx layout: value for slot j of tile t lives at
        # [16 rows](j%16), col t*8 + j//16, replicated over 8 groups of 16
        flat = np.zeros((ttot, P), dtype=np.int16)
        flat[tno, pno] = lcol
        wrapped = flat.reshape(ttot, 8, 16).transpose(2, 0, 1).reshape(16, ttot * 8)
        idx_all[m] = np.tile(wrapped, (8, 1))
        dlf_all[m][tno, pno] = (dl & 127).astype(np.float32)

    # per-tile destination window (32-aligned; union over cores)
    valid = dlf_all >= 0
    gmin = np.where(valid, dlf_all, 128.0).min(axis=(0, 2))
    gmax = np.where(valid, dlf_all, -1.0).max(axis=(0, 2))
    gmin = np.minimum(gmin, gmax.clip(0))  # empty tile -> [0, 0]
    lo32 = (gmin.astype(np.int64) // 32) * 32
    fits32 = (gmax < lo32 + 32) & (lo32 < 96)  # base partition 96 not encodable
    fits64a = gmax < 64
    fits64b = gmin >= 64
    win_w = np.where(fits32, 32, np.where(fits64a | fits64b, 64, 128)).astype(np.int64)
    win_lo = np.where(
        fits32, lo32, np.where(fits64a, 0, np.where(fits64b, 64, 0))
    ).astype(np.int64)
    dl_all = np.empty((NCORES, P, ttot), dtype=np.float32)
    for m in range(NCORES):
        rel = dlf_all[m] - win_lo[:, None]
        rel[~valid[m]] = -1.0
        dl_all[m] = rel.T

    # ---- dest-major degree layout for cc ------------------------------------
    # per block: K(b) = 1 + cross-core max degree in block; slot 0 = own node
    deg_pad = np.zeros((NCORES, npc_pad), dtype=np.int64)
    for m in range(NCORES):
        deg_pad[m, :npc] = deg[m * npc : (m + 1) * npc]
    Kb = deg_pad.reshape(NCORES, nblk, P).max(axis=(0, 2)) + 1
    koff = np.zeros(nblk + 1, dtype=np.int64)
    np.cumsum(Kb, out=koff[1:])
    ktot = int(koff[-1])
    degE_all = np.full((NCORES, P, ktot), 1e30, dtype=np.float32)
    for m in range(NCORES):
        lo = rowstart[m * npc]
        for bi in range(nblk):
            K = int(Kb[bi])
            ids = m * npc + bi * P + np.arange(P)
            valid = ids < (m + 1) * npc
            idc = np.where(valid, ids, m * npc)
            dg = deg[idc]
            seg = np.full((P, K), 1e30, dtype=np.float32)
            seg[:, 0] = np.where(valid, dg, 1e30).astype(np.float32)
            kg = np.arange(K - 1, dtype=np.int64)[None, :]
            gi = rowstart[idc][:, None] + kg
            ok = (kg < dg[:, None]) & valid[:, None]
            src_deg = deg[col_s[np.minimum(gi, len(col_s) - 1)]]
            seg[:, 1:] = np.where(ok, src_deg, 1e30).astype(np.float32)
            degE_all[m, :, int(koff[bi]) : int(koff[bi + 1])] = seg

    # ---- launch-1 inputs ----------------------------------------------------
    x_sh = np.zeros((NCORES, npc_pad, din), dtype=np.float32)
    deg_sh = np.ones((NCORES, P, npc_pad // P), dtype=np.float32)
    xf = np.asarray(x, dtype=np.float32)
    for m in range(NCORES):
        x_sh[m, :npc] = xf[m * npc : (m + 1) * npc]
        dm = np.ones(npc_pad, dtype=np.float32)
        dm[:npc] = deg[m * npc : (m + 1) * npc].astype(np.float32)
        deg_sh[m] = dm.reshape(npc_pad // P, P).T

    meta = dict(
        N=N, din=din, dout=dout, npc=npc, nblk=nblk, npc_pad=npc_pad,
        nchunk=nchunk, n_y=n_y, ttot=ttot, ktot=ktot,
        koff=koff, sblocks=sblocks, sb_tiles=sb_tiles, sb_calls=sb_calls,
        blk_runs=blk_runs, sb_of_blk=sb_of_blk, tile_base=tile_base,
        win_lo=win_lo, win_w=win_w,
    )
    data = dict(
        idx_all=idx_all, dl_all=dl_all, degE_all=degE_all,
        x_sh=x_sh, deg_sh=deg_sh,
    )
    return meta, data


def kernel(x, edge_index, W, b):
    x = np.asarray(x, dtype=np.float32)
    W = np.asarray(W, dtype=np.float32)
    b = np.asarray(b, dtype=np.float32)
    edge_index = np.asarray(edge_index)
    meta, data = _prep(x, edge_index, W, b)
    N, din, dout = meta["N"], meta["din"], meta["dout"]

    key1 = ("l1", meta["npc_pad"], din)
    if key1 not in _cache:
        _cache[key1] = _build_launch1(meta["npc_pad"], din)
    nc1 = _cache[key1]

    in_maps1 = [
        {"x_sh": data["x_sh"][m], "deg_sh": data["deg_sh"][m]} for m in range(NCORES)
    ]
    res1 = run_bass_kernel_spmd(nc1, in_maps1, list(range(NCORES))).results
    import ml_dtypes
    bf16 = np.dtype(ml_dtypes.bfloat16)
    y_full = np.zeros((meta["n_y"], 2 * din), dtype=bf16)
    for m in range(NCORES):
        y_full[m * meta["npc"] : (m + 1) * meta["npc"]] = res1[m]["y_sh"][: meta["npc"]]

    key2 = (
        "l2", N, din, dout,
        tuple(int(t) for t in np.asarray(meta["sb_tiles"])),
        meta["ttot"], meta["ktot"],
        tuple(int(v) for v in meta["win_lo"]),
        tuple(int(v) for v in meta["win_w"]),
    )
    if key2 not in _cache:
        _cache[key2] = _build_launch2(meta)
    nc2 = _cache[key2]

    wt = np.ascontiguousarray(W.T)
    brep = np.repeat(b[None, :], P, axis=0).astype(np.float32)
    ys_all = np.zeros((NCORES, meta["npc_pad"], 2 * din), dtype=bf16)
    for m in range(NCORES):
        ys_all[m, : meta["npc"]] = y_full[m * meta["npc"] : (m + 1) * meta["npc"]]
    in_maps2 = [
        {
            "y_t": y_full,
            "ys_t": ys_all[m],
            "idx_t": data["idx_all"][m],
            "dl_t": data["dl_all"][m],
            "degE_t": data["degE_all"][m],
            "wt_t": wt,
            "brep_t": brep,
        }
        for m in range(NCORES)
    ]
    res2 = run_bass_kernel_spmd(nc2, in_maps2, list(range(NCORES))).results

    LAST.clear()
    LAST.update(nc1=nc1, in_maps1=in_maps1, nc2=nc2, in_maps2=in_maps2)

    out = np.empty((N, dout), dtype=np.float32)
    for m in range(NCORES):
        out[m * meta["npc"] : (m + 1) * meta["npc"]] = res2[m]["out_t"][: meta["npc"]]
    return out



# revision 6
# speedup vs baseline: 90.2951x; 90.2951x over previous
"""GCN layer kernel for Trainium2, 8 NeuronCores — single launch.

Math (identical to reference):
    deg = bincount(row);  d = 1/sqrt(deg)
    h   = x @ W.T + b
    out = d * segment_sum(d[col] * h[col], row) + d^2 * h

Aggregate-then-transform (linear map commutes with the segment sum):
    y[j]   = d_j * x_j                      (host, bf16)
    U[r]   = sum_{edges (r,c)} y[c] + y[r]  (self term = extra edge slot)
    cc[r]  = sum_{edges (r,c)} d_c + d_r    (host)
    out[r] = d_r * (U[r] @ W.T + cc[r] * b)

Device program (SPMD over 8 cores, destinations sharded):
  * destinations are dealt to (core, position) round-robin in descending
    degree order, which equalizes per-core work per gather call and so
    minimizes the cross-core-max padding the static SPMD schedule needs.
  * edges sorted by (dest superblock of SBD, source chunk, dest); bulk
    gathered with gpsimd.dma_gather (256B bf16 y rows).  Gathered edge i
    lands at SBUF partition i%128, tile i//128; padding only at the end of
    each gather call, with idx=-1 (HW skips trailing negative indices).
  * per 128-edge tile a 0/1 selection matrix S[edge, dest_local] is built
    with one is_equal against an iota row over the tile's dest window, and
    one PE matmul accumulates G^T @ S into the 512-dest half's PSUM bank as
    U^T [feat, dest] — the final W matmul reads that as lhsT directly, so
    there are no transposes anywhere.  Tiles straddling a half boundary get
    one windowed matmul per half.
  * per 128-dest stripe: o2 = U_T_stripe^T @ W^T plus a rank-1 cc x b
    matmul into the same PSUM, then one activation copy scaled by d -> out.
"""

import numpy as np
import sys

sys.path.insert(0, "/opt/trn_rl_repo")

import concourse.bacc as bacc
import concourse.tile as tile
from concourse import mybir
from concourse.bass_utils import run_bass_kernel_spmd

NCORES = 8
P = 128
MAXCHUNK = 32000  # dma_gather idx is int16: source chunks must stay < 32768 rows
SBD = 1024  # dests per superblock (gather-slab granularity)
HW = 512  # dests per PSUM half (one 2KB fp32 bank)
F32 = mybir.dt.float32
I16 = mybir.dt.int16
BF16 = mybir.dt.bfloat16

_cache = {}
LAST = {}  # populated on each kernel() call (for profiling in test.py)


def _build(meta, nrep=1):
    din = meta["din"]
    dout = meta["dout"]
    chunk = meta["chunk"]
    n_y = meta["n_y"]
    npc_pad = meta["npc_pad"]
    nblk = meta["nblk"]  # 128-dest stripes per core
    nsb = meta["nsb"]
    nhalf = meta["nhalf"]
    ttot = meta["ttot"]
    sb_calls = meta["sb_calls"]  # per sb: list of (chunk, tile_off_in_sb, ntiles)
    sb_base = meta["sb_base"]  # per sb: global tile offset
    max_sb_tiles = meta["max_sb_tiles"]
    half_tiles = meta["half_tiles"]  # per half: list of (tile_in_sb, lo, w)
    sbw = meta["sbw"]  # iota width (SBD)

    nc = bacc.Bacc(
        "TRN2",
        target_bir_lowering=False,
        debug=False,
        enable_asserts=False,
        num_devices=NCORES,
    )
    y_t = nc.dram_tensor("y_t", [n_y, din], BF16, kind="ExternalInput").ap()
    idx_t = nc.dram_tensor("idx_t", [P, ttot * 8], I16, kind="ExternalInput").ap()
    dl_t = nc.dram_tensor("dl_t", [P, ttot], F32, kind="ExternalInput").ap()
    wt_t = nc.dram_tensor("wt_t", [din, dout], F32, kind="ExternalInput").ap()
    brow_t = nc.dram_tensor("brow_t", [1, dout], F32, kind="ExternalInput").ap()
    ccrow_t = nc.dram_tensor("ccrow_t", [1, npc_pad], F32, kind="ExternalInput").ap()
    dsb_t = nc.dram_tensor("dsb_t", [P, nblk], F32, kind="ExternalInput").ap()
    out_t = nc.dram_tensor("out_t", [npc_pad, dout], F32, kind="ExternalOutput").ap()
    out_v = out_t.rearrange("(t p) f -> p t f", p=P)

    with tile.TileContext(nc) as tc:
        with (
            tc.tile_pool(name="const", bufs=1) as cpool,
            tc.tile_pool(name="slab", bufs=2) as gpool,
            tc.tile_pool(name="sel", bufs=6) as selpool,
            tc.tile_pool(name="work", bufs=3) as wpool,
            tc.tile_pool(name="out", bufs=2) as opool,
            tc.tile_pool(name="psU", bufs=3, space="PSUM") as ppool,
            tc.tile_pool(name="psO", bufs=2, space="PSUM") as p2pool,
        ):
            iota_i = cpool.tile([P, sbw], dtype=mybir.dt.int32)
            nc.gpsimd.iota(iota_i[:], pattern=[[1, sbw]], base=0, channel_multiplier=0)
            iota_h = cpool.tile([P, sbw], dtype=mybir.dt.float16)
            nc.vector.tensor_copy(iota_h[:], iota_i[:])
            wt_sb = cpool.tile([din, dout], dtype=F32)
            nc.sync.dma_start(out=wt_sb[:], in_=wt_t[:, :])
            brow_sb = cpool.tile([1, dout], dtype=F32)
            nc.sync.dma_start(out=brow_sb[:], in_=brow_t[:, :])
            ccrow_sb = cpool.tile([1, npc_pad], dtype=F32)
            nc.sync.dma_start(out=ccrow_sb[:], in_=ccrow_t[:, :])
            dsb_sb = cpool.tile([P, nblk], dtype=F32)
            nc.sync.dma_start(out=dsb_sb[:], in_=dsb_t[:, :])
            idx_sb = cpool.tile([P, ttot * 8], dtype=I16)
            nc.sync.dma_start(out=idx_sb[:], in_=idx_t[:, :])
            dl_sb = cpool.tile([P, ttot], dtype=F32)
            nc.sync.dma_start(out=dl_sb[:], in_=dl_t[:, :])

            def body():
                for sb in range(nsb):
                    tb = sb_base[sb]
                    slab = gpool.tile([P, max_sb_tiles, din], dtype=BF16, tag="slab")
                    for (c, toff, nt) in sb_calls[sb]:
                        c_lo = c * chunk
                        c_hi = min((c + 1) * chunk, n_y)
                        nc.gpsimd.dma_gather(
                            out_ap=slab[:, toff : toff + nt, :],
                            in_ap=y_t[c_lo:c_hi, :],
                            idxs_ap=idx_sb[:, (tb + toff) * 8 : (tb + toff + nt) * 8],
                            num_idxs=nt * P,
                            num_idxs_reg=nt * P,
                            elem_size=din,
                            single_packet=False,
                        )
                    for hpar in range(SBD // HW):
                        h = sb * (SBD // HW) + hpar
                        if h >= nhalf:
                            break
                        tiles = half_tiles[h]
                        cw = min(HW, npc_pad - h * HW)
                        U = ppool.tile([P, HW], dtype=F32, space="PSUM", tag="U")
                        ntb = len(tiles)
                        for ti, (tloc, lo, w) in enumerate(tiles):
                            tg = tb + tloc
                            if ti == 0:
                                lo, w = 0, cw  # full-width: clears the bank
                            st = selpool.tile([P, HW], dtype=BF16, tag="st")
                            nc.vector.tensor_scalar(
                                out=st[:, 0:w],
                                in0=iota_h[:, hpar * HW + lo : hpar * HW + lo + w],
                                scalar1=dl_sb[:, tg : tg + 1],
                                scalar2=None,
                                op0=mybir.AluOpType.is_equal,
                            )
                            nc.tensor.matmul(
                                out=U[:, lo : lo + w],
                                lhsT=slab[:, tloc, :],
                                rhs=st[:, 0:w],
                                start=(ti == 0),
                                stop=(ti == ntb - 1),
                            )
                        ut = wpool.tile([P, HW], dtype=F32, tag="ut")
                        nc.scalar.activation(
                            ut[:], U[:], mybir.ActivationFunctionType.Copy
                        )
                        nq = min(HW // P, nblk - h * (HW // P))
                        ob = opool.tile([P, HW // P, dout], dtype=F32, tag="ob")
                        for q in range(nq):
                            g = h * (HW // P) + q
                            o2 = p2pool.tile([P, dout], dtype=F32, space="PSUM", tag="o2")
                            nc.tensor.matmul(
                                out=o2[:],
                                lhsT=ut[:, q * P : (q + 1) * P],
                                rhs=wt_sb[:],
                                start=True,
                                stop=False,
                            )
                            nc.tensor.matmul(
                                out=o2[:],
                                lhsT=ccrow_sb[0:1, g * P : (g + 1) * P],
                                rhs=brow_sb[0:1, :],
                                start=False,
                                stop=True,
                            )
                            nc.scalar.activation(
                                ob[:, q, :],
                                o2[:],
                                mybir.ActivationFunctionType.Copy,
                                scale=dsb_sb[:, g : g + 1],
                            )
                        nc.sync.dma_start(
                            out=out_v[:, h * (HW // P) : h * (HW // P) + nq, :],
                            in_=ob[:, 0:nq, :],
                        )

            if nrep > 1:
                with tc.For_i(0, nrep, 1):
                    body()
            else:
                body()
    nc.compile()
    return nc


def _prep(x, edge_index, W, b):
    import ml_dtypes

    bf16 = np.dtype(ml_dtypes.bfloat16)
    N, din = x.shape
    dout = W.shape[0]
    npc = N // NCORES
    assert npc * NCORES == N
    nblk = (npc + P - 1) // P
    npc_pad = nblk * P
    nhalf = (npc_pad + HW - 1) // HW
    hpb = SBD // HW
    nsb = (nhalf + hpb - 1) // hpb
    sbw = min(SBD, npc_pad)
    nchunk = (N + MAXCHUNK - 1) // MAXCHUNK
    chunk = (N + nchunk - 1) // nchunk
    n_y = N

    row = np.asarray(edge_index[0], dtype=np.int64)
    col = np.asarray(edge_index[1], dtype=np.int64)
    deg = np.bincount(row, minlength=N)
    d = 1.0 / np.sqrt(deg.astype(np.float64))
    y = (x.astype(np.float64) * d[:, None]).astype(np.float32).astype(bf16)
    cc = d + np.bincount(row, weights=d[col], minlength=N)

    # destination -> (core, position): deal in descending-degree order so all
    # cores see near-identical per-call work (minimizes cross-core-max pad).
    perm = np.argsort(-deg, kind="stable")  # perm[rank] = node
    rank = np.empty(N, dtype=np.int64)
    rank[perm] = np.arange(N)

    # ---- slots = edges + self edges, dest-sharded -------------------------
    rows_a = np.concatenate([row, np.arange(N, dtype=np.int64)])
    cols_a = np.concatenate([col, np.arange(N, dtype=np.int64)])
    r = rank[rows_a]
    core = r % NCORES
    rl = r // NCORES  # dest position within core, 0..npc-1
    sb = rl // SBD
    ch = cols_a // chunk
    sbg = core * nsb + sb
    order = np.lexsort((rl, ch, sbg))
    core_s, rl_s, ch_s, cols_s = core[order], rl[order], ch[order], cols_a[order]
    sb_s = rl_s // SBD
    gid = (core_s * nsb + sb_s) * nchunk + ch_s
    ngrp = NCORES * nsb * nchunk
    gcnt = np.bincount(gid, minlength=ngrp).reshape(NCORES, nsb * nchunk)
    tcnt = (gcnt.max(axis=0) + P - 1) // P  # [nsb*nchunk]
    tile_start = np.zeros(nsb * nchunk + 1, dtype=np.int64)
    np.cumsum(tcnt, out=tile_start[1:])
    ttot = int(tile_start[-1])
    sb_base = [int(tile_start[s * nchunk]) for s in range(nsb)]
    sb_tiles = [
        int(tile_start[(s + 1) * nchunk] - tile_start[s * nchunk]) for s in range(nsb)
    ]
    max_sb_tiles = max(sb_tiles)

    sb_calls = []
    for s in range(nsb):
        calls = []
        for c in range(nchunk):
            nt = int(tcnt[s * nchunk + c])
            if nt:
                calls.append((c, int(tile_start[s * nchunk + c]) - sb_base[s], nt))
        sb_calls.append(calls)

    # ---- per-core slot data ----------------------------------------------
    grp_start = np.zeros(ngrp + 1, dtype=np.int64)
    np.cumsum(np.bincount(gid, minlength=ngrp), out=grp_start[1:])
    rank_in_g = np.arange(len(gid), dtype=np.int64) - grp_start[gid]
    gnc = gid % (nsb * nchunk)
    slot = tile_start[gnc] * P + rank_in_g
    tno = slot // P
    pno = slot - tno * P
    lidx = (cols_s - ch_s * chunk).astype(np.int16)
    dl10 = rl_s - sb_s * SBD  # 0..SBD-1 within superblock

    idx_all = np.zeros((NCORES, P, ttot * 8), dtype=np.int16)
    dl_all = np.empty((NCORES, P, ttot), dtype=np.float32)
    nkey = ttot * hpb
    wmin = np.full(nkey, SBD, dtype=np.int64)
    wmax = np.full(nkey, -1, dtype=np.int64)
    for m in range(NCORES):
        sel = core_s == m
        flat = np.zeros((ttot, P), dtype=np.int16)
        flat[tno[sel], pno[sel]] = lidx[sel]
        wrapped = flat.reshape(ttot, 8, 16).transpose(2, 0, 1).reshape(16, ttot * 8)
        idx_all[m] = np.tile(wrapped, (8, 1))
        fdl = np.full((ttot, P), -1.0, dtype=np.float32)
        fdl[tno[sel], pno[sel]] = dl10[sel].astype(np.float32)
        dl_all[m] = fdl.T
        key = tno[sel] * hpb + dl10[sel] // HW
        np.minimum.at(wmin, key, dl10[sel])
        np.maximum.at(wmax, key, dl10[sel])

    half_tiles = [[] for _ in range(nhalf)]
    for s in range(nsb):
        for c in range(nchunk):
            t0, t1 = int(tile_start[s * nchunk + c]), int(tile_start[s * nchunk + c + 1])
            for t in range(t0, t1):
                for hp in range(hpb):
                    h = s * hpb + hp
                    if h >= nhalf:
                        break
                    k = t * hpb + hp
                    if wmax[k] < 0:
                        continue
                    lo = int(wmin[k]) - hp * HW
                    w = int(wmax[k]) - hp * HW + 1 - lo
                    half_tiles[h].append((t - sb_base[s], lo, w))
    for h in range(nhalf):
        assert half_tiles[h], f"half {h} has no tiles"

    # ---- small per-core arrays (permuted node order) ----------------------
    ccrow = np.zeros((NCORES, 1, npc_pad), dtype=np.float32)
    dsb = np.ones((NCORES, P, nblk), dtype=np.float32)
    pos = np.arange(npc, dtype=np.int64)
    for m in range(NCORES):
        nodes = perm[pos * NCORES + m]
        ccrow[m, 0, :npc] = cc[nodes]
        dm = np.ones(npc_pad, dtype=np.float64)
        dm[:npc] = d[nodes]
        dsb[m] = dm.reshape(nblk, P).T.astype(np.float32)

    meta = dict(
        N=N, din=din, dout=dout, npc=npc, nblk=nblk, npc_pad=npc_pad,
        nchunk=nchunk, chunk=chunk, n_y=n_y, nsb=nsb, nhalf=nhalf, ttot=ttot,
        sb_calls=sb_calls, sb_base=sb_base, max_sb_tiles=max_sb_tiles,
        half_tiles=half_tiles, sbw=sbw,
    )
    data = dict(y=y, idx_all=idx_all, dl_all=dl_all, ccrow=ccrow, dsb=dsb, perm=perm)
    return meta, data


def _meta_key(meta):
    return (
        meta["N"], meta["din"], meta["dout"], meta["ttot"], meta["max_sb_tiles"],
        tuple(tuple(t) for tl in meta["half_tiles"] for t in tl),
        tuple(tuple(c) for cl in meta["sb_calls"] for c in cl),
    )


def kernel(x, edge_index, W, b):
    x = np.asarray(x, dtype=np.float32)
    W = np.asarray(W, dtype=np.float32)
    b = np.asarray(b, dtype=np.float32)
    edge_index = np.asarray(edge_index)
    meta, data = _prep(x, edge_index, W, b)
    N, din, dout = meta["N"], meta["din"], meta["dout"]

    key = ("l", _meta_key(meta))
    if key not in _cache:
        _cache[key] = _build(meta)
    nc = _cache[key]

    wt = np.ascontiguousarray(W.T)
    brow = b[None, :].astype(np.float32)
    in_maps = [
        {
            "y_t": data["y"],
            "idx_t": data["idx_all"][m],
            "dl_t": data["dl_all"][m],
            "wt_t": wt,
            "brow_t": brow,
            "ccrow_t": data["ccrow"][m],
            "dsb_t": data["dsb"][m],
        }
        for m in range(NCORES)
    ]
    res = run_bass_kernel_spmd(nc, in_maps, list(range(NCORES))).results

    LAST.clear()
    LAST.update(nc=nc, in_maps=in_maps, meta=meta)

    out = np.empty((N, dout), dtype=np.float32)
    perm = data["perm"]
    pos = np.arange(meta["npc"], dtype=np.int64)
    for m in range(NCORES):
        out[perm[pos * NCORES + m]] = res[m]["out_t"][: meta["npc"]]
    return out


# revision 15
# speedup vs baseline: 311.6819x; 3.4518x over previous
"""GCN layer kernel for Trainium2, 8 NeuronCores — single launch.

Math (identical to reference):
    deg = bincount(row);  d = 1/sqrt(deg)
    h   = x @ W.T + b
    out = d * segment_sum(d[col] * h[col], row) + d^2 * h

Aggregate-then-transform (linear map commutes with the segment sum):
    y[j]   = d_j * x_j                      (host, bf16)
    U[r]   = sum_{edges (r,c)} y[c] + y[r]  (self term = extra edge slot)
    cc[r]  = sum_{edges (r,c)} d_c + d_r    (host)
    out[r] = d_r * (U[r] @ W.T + cc[r] * b)

Device program (SPMD over 8 cores, destinations sharded):
  * destinations are dealt to (core, position) round-robin in descending
    degree order, which equalizes per-core work per gather call and so
    minimizes the cross-core-max padding the static SPMD schedule needs.
  * edges sorted by (dest superblock of SBD, source chunk, dest); bulk
    gathered with gpsimd.dma_gather (256B bf16 y rows) spread round-robin
    over 4 SWDGE queues (a single queue runs at only ~27 GB/s).  Gathered
    edge i lands at SBUF partition i%128, tile i//128.
  * per 128-edge tile a 0/1 selection matrix S[edge, dest_local] over the
    tile's dest window is PRECOMPUTED ON HOST (bf16) and streamed in via
    regular DMA (~20 MB/core; cheaper than building it on DVE, which costs
    ~290 ns/instruction on HW).  One PE matmul per tile accumulates
    G^T @ S into the 512-dest half's PSUM bank as U^T [feat, dest] — the
    final W matmul reads that as lhsT directly, so there are no transposes.
  * per 128-dest stripe: o2 = U_T_stripe^T @ W^T plus a rank-1 cc x b
    matmul into the same PSUM, then one activation copy scaled by d -> out.
"""

import numpy as np
import sys

sys.path.insert(0, "/opt/trn_rl_repo")

import concourse.bacc as bacc
import concourse.tile as tile
from concourse import mybir
from concourse.bass_utils import run_bass_kernel_spmd

NCORES = 8
P = 128
MAXCHUNK = 32000  # dma_gather idx is int16: source chunks must stay < 32768 rows
SBD = 1024  # dests per superblock (gather-slab granularity)
HW = 512  # dests per PSUM half (one 2KB fp32 bank)
NQUEUES = 4
F32 = mybir.dt.float32
I16 = mybir.dt.int16
BF16 = mybir.dt.bfloat16

_cache = {}
LAST = {}  # populated on each kernel() call (for profiling in test.py)


def _build(meta, nrep=1, mode="full"):
    din = meta["din"]
    dout = meta["dout"]
    chunk = meta["chunk"]
    n_y = meta["n_y"]
    npc_pad = meta["npc_pad"]
    nblk = meta["nblk"]  # 128-dest stripes per core
    nsb = meta["nsb"]
    nhalf = meta["nhalf"]
    ttot = meta["ttot"]
    stot = meta["stot"]
    sb_calls = meta["sb_calls"]  # per sb: list of (chunk, tile_off_in_sb, ntiles)
    sb_base = meta["sb_base"]  # per sb: global tile offset
    s_base = meta["s_base"]  # per sb: S column offset
    sw_sb = meta["sw_sb"]  # per sb: S columns
    max_sb_tiles = meta["max_sb_tiles"]
    sb_tiles = meta["sb_tiles"]
    max_sw = meta["max_sw"]
    half_tiles = meta["half_tiles"]  # per half: list of (tile_in_sb, lo, w, soff_in_sb)

    nc = bacc.Bacc(
        "TRN2",
        target_bir_lowering=False,
        debug=False,
        enable_asserts=False,
        num_devices=NCORES,
        num_swdge_queues=NQUEUES,
    )
    y_t = nc.dram_tensor("y_t", [n_y, din], BF16, kind="ExternalInput").ap()
    idx_t = nc.dram_tensor("idx_t", [P, ttot * 8], I16, kind="ExternalInput").ap()
    s_t = nc.dram_tensor("s_t", [P, stot], BF16, kind="ExternalInput").ap()
    wt_t = nc.dram_tensor("wt_t", [din, dout], F32, kind="ExternalInput").ap()
    brow_t = nc.dram_tensor("brow_t", [1, dout], BF16, kind="ExternalInput").ap()
    ccrow_t = nc.dram_tensor("ccrow_t", [1, npc_pad], BF16, kind="ExternalInput").ap()
    dsb_t = nc.dram_tensor("dsb_t", [P, nblk], F32, kind="ExternalInput").ap()
    out_t = nc.dram_tensor("out_t", [npc_pad, dout], F32, kind="ExternalOutput").ap()
    out_v = out_t.rearrange("(t p) f -> p t f", p=P)

    with tile.TileContext(nc) as tc:
        with (
            tc.tile_pool(name="const", bufs=1) as cpool,
            tc.tile_pool(name="slab", bufs=2) as gpool,
            tc.tile_pool(name="sslab", bufs=2) as spool,
            tc.tile_pool(name="idxp", bufs=2) as ipool,
            tc.tile_pool(name="work", bufs=3) as wpool,
            tc.tile_pool(name="out", bufs=2) as opool,
            tc.tile_pool(name="psU", bufs=3, space="PSUM") as ppool,
            tc.tile_pool(name="psO", bufs=2, space="PSUM") as p2pool,
        ):
            wt_sb = cpool.tile([din, dout], dtype=F32)
            nc.sync.dma_start(out=wt_sb[:], in_=wt_t[:, :])
            brow_sb = cpool.tile([1, dout], dtype=BF16)
            nc.sync.dma_start(out=brow_sb[:], in_=brow_t[:, :])
            ccrow_sb = cpool.tile([1, npc_pad], dtype=BF16)
            nc.sync.dma_start(out=ccrow_sb[:], in_=ccrow_t[:, :])
            dsb_sb = cpool.tile([P, nblk], dtype=F32)
            nc.sync.dma_start(out=dsb_sb[:], in_=dsb_t[:, :])
            zs = cpool.tile([P, HW], dtype=BF16)
            nc.gpsimd.memset(zs[:], 0.0)

            def body():
                qrr = [0]
                for sb in range(nsb):
                    tb = sb_base[sb]
                    nt_sb = sb_tiles[sb]
                    idx_sb = ipool.tile([P, max_sb_tiles * 8], dtype=I16, tag="idx")
                    nc.sync.dma_start(
                        out=idx_sb[:, 0 : nt_sb * 8],
                        in_=idx_t[:, tb * 8 : (tb + nt_sb) * 8],
                    )
                    slab = gpool.tile([P, max_sb_tiles, din], dtype=BF16, tag="slab")
                    for (c, toff, nt) in sb_calls[sb] if mode != "nogather" else []:
                        c_lo = c * chunk
                        c_hi = min((c + 1) * chunk, n_y)
                        nc.gpsimd.dma_gather(
                            out_ap=slab[:, toff : toff + nt, :],
                            in_ap=y_t[c_lo:c_hi, :],
                            idxs_ap=idx_sb[:, toff * 8 : (toff + nt) * 8],
                            num_idxs=nt * P,
                            num_idxs_reg=nt * P,
                            elem_size=din,
                            single_packet=False,
                            queue_num=qrr[0] % NQUEUES,
                        )
                        qrr[0] += 1
                    sslab = spool.tile([P, max_sw], dtype=BF16, tag="sslab")
                    nc.sync.dma_start(
                        out=sslab[:, 0 : sw_sb[sb]],
                        in_=s_t[:, s_base[sb] : s_base[sb] + sw_sb[sb]],
                    )
                    for hpar in range(SBD // HW):
                        h = sb * (SBD // HW) + hpar
                        if h >= nhalf:
                            break
                        if mode == "gather":
                            nq = min(HW // P, nblk - h * (HW // P))
                            ob = opool.tile([P, HW // P, dout], dtype=F32, tag="ob")
                            for q in range(nq):
                                nc.scalar.activation(
                                    ob[:, q, :], wt_sb[:],
                                    mybir.ActivationFunctionType.Copy,
                                )
                            nc.sync.dma_start(
                                out=out_v[:, h * (HW // P) : h * (HW // P) + nq, :],
                                in_=ob[:, 0:nq, :],
                            )
                            continue
                        tiles = half_tiles[h]
                        cw = min(HW, npc_pad - h * HW)
                        U = ppool.tile([P, HW], dtype=F32, space="PSUM", tag="U")
                        ntb = len(tiles)
                        nc.tensor.matmul(
                            out=U[:, 0:cw],
                            lhsT=slab[:, 0, :],
                            rhs=zs[:, 0:cw],
                            start=True,
                            stop=False,
                        )
                        for ti, (tloc, lo, w, soff) in enumerate(tiles):
                            nc.tensor.matmul(
                                out=U[:, lo : lo + w],
                                lhsT=slab[:, tloc, :],
                                rhs=sslab[:, soff : soff + w],
                                start=False,
                                stop=(ti == ntb - 1),
                            )
                        ut = wpool.tile([P, HW], dtype=F32, tag="ut")
                        nc.scalar.activation(
                            ut[:], U[:], mybir.ActivationFunctionType.Copy
                        )
                        nq = min(HW // P, nblk - h * (HW // P))
                        ob = opool.tile([P, HW // P, dout], dtype=F32, tag="ob")
                        for q in range(nq):
                            g = h * (HW // P) + q
                            o2 = p2pool.tile([P, dout], dtype=F32, space="PSUM", tag="o2")
                            nc.tensor.matmul(
                                out=o2[:],
                                lhsT=ut[:, q * P : (q + 1) * P],
                                rhs=wt_sb[:],
                                start=True,
                                stop=False,
                            )
                            nc.tensor.matmul(
                                out=o2[:],
                                lhsT=ccrow_sb[0:1, g * P : (g + 1) * P],
                                rhs=brow_sb[0:1, :],
                                start=False,
                                stop=True,
                            )
                            nc.scalar.activation(
                                ob[:, q, :],
                                o2[:],
                                mybir.ActivationFunctionType.Copy,
                                scale=dsb_sb[:, g : g + 1],
                            )
                        nc.sync.dma_start(
                            out=out_v[:, h * (HW // P) : h * (HW // P) + nq, :],
                            in_=ob[:, 0:nq, :],
                        )

            if nrep > 1:
                with tc.For_i(0, nrep, 1):
                    body()
            else:
                body()
    nc.compile()
    return nc


def _prep(x, edge_index, W, b):
    import ml_dtypes

    bf16 = np.dtype(ml_dtypes.bfloat16)
    N, din = x.shape
    dout = W.shape[0]
    npc = N // NCORES
    assert npc * NCORES == N
    nblk = (npc + P - 1) // P
    npc_pad = nblk * P
    nhalf = (npc_pad + HW - 1) // HW
    hpb = SBD // HW
    nsb = (nhalf + hpb - 1) // hpb
    nchunk = (N + MAXCHUNK - 1) // MAXCHUNK
    chunk = (N + nchunk - 1) // nchunk
    n_y = N

    row = np.asarray(edge_index[0], dtype=np.int64)
    col = np.asarray(edge_index[1], dtype=np.int64)
    deg = np.bincount(row, minlength=N)
    d = 1.0 / np.sqrt(deg.astype(np.float64))
    y = (x.astype(np.float64) * d[:, None]).astype(np.float32).astype(bf16)
    cc = d + np.bincount(row, weights=d[col], minlength=N)

    # destination -> (core, position): deal in descending-degree order so all
    # cores see near-identical per-call work (minimizes cross-core-max pad).
    perm = np.argsort(-deg, kind="stable")  # perm[rank] = node
    rank = np.empty(N, dtype=np.int64)
    rank[perm] = np.arange(N)

    # ---- slots = edges + self edges, dest-sharded -------------------------
    rows_a = np.concatenate([row, np.arange(N, dtype=np.int64)])
    cols_a = np.concatenate([col, np.arange(N, dtype=np.int64)])
    r = rank[rows_a]
    core = r % NCORES
    rl = r // NCORES  # dest position within core, 0..npc-1
    sb = rl // SBD
    ch = cols_a // chunk
    sbg = core * nsb + sb
    order = np.lexsort((rl, ch, sbg))
    core_s, rl_s, ch_s, cols_s = core[order], rl[order], ch[order], cols_a[order]
    sb_s = rl_s // SBD
    gid = (core_s * nsb + sb_s) * nchunk + ch_s
    ngrp = NCORES * nsb * nchunk
    gcnt = np.bincount(gid, minlength=ngrp).reshape(NCORES, nsb * nchunk)
    tcnt = (gcnt.max(axis=0) + P - 1) // P  # [nsb*nchunk]
    tile_start = np.zeros(nsb * nchunk + 1, dtype=np.int64)
    np.cumsum(tcnt, out=tile_start[1:])
    ttot = int(tile_start[-1])
    sb_base = [int(tile_start[s * nchunk]) for s in range(nsb)]
    sb_tiles = [
        int(tile_start[(s + 1) * nchunk] - tile_start[s * nchunk]) for s in range(nsb)
    ]
    max_sb_tiles = max(sb_tiles)

    sb_calls = []
    for s in range(nsb):
        calls = []
        for c in range(nchunk):
            nt = int(tcnt[s * nchunk + c])
            if nt:
                calls.append((c, int(tile_start[s * nchunk + c]) - sb_base[s], nt))
        sb_calls.append(calls)

    # ---- per-core slot data ----------------------------------------------
    grp_start = np.zeros(ngrp + 1, dtype=np.int64)
    np.cumsum(np.bincount(gid, minlength=ngrp), out=grp_start[1:])
    rank_in_g = np.arange(len(gid), dtype=np.int64) - grp_start[gid]
    gnc = gid % (nsb * nchunk)
    slot = tile_start[gnc] * P + rank_in_g
    tno = slot // P
    pno = slot - tno * P
    lidx = (cols_s - ch_s * chunk).astype(np.int16)
    dl10 = rl_s - sb_s * SBD  # 0..SBD-1 within superblock

    idx_all = np.zeros((NCORES, P, ttot * 8), dtype=np.int16)
    dl_all = np.full((NCORES, ttot, P), -1.0, dtype=np.float32)
    nkey = ttot * hpb
    wmin = np.full(nkey, SBD, dtype=np.int64)
    wmax = np.full(nkey, -1, dtype=np.int64)
    for m in range(NCORES):
        sel = core_s == m
        flat = np.zeros((ttot, P), dtype=np.int16)
        flat[tno[sel], pno[sel]] = lidx[sel]
        wrapped = flat.reshape(ttot, 8, 16).transpose(2, 0, 1).reshape(16, ttot * 8)
        idx_all[m] = np.tile(wrapped, (8, 1))
        dl_all[m][tno[sel], pno[sel]] = dl10[sel].astype(np.float32)
        key = tno[sel] * hpb + dl10[sel] // HW
        np.minimum.at(wmin, key, dl10[sel])
        np.maximum.at(wmax, key, dl10[sel])

    # ---- per-half tile schedule + packed host-built S ---------------------
    half_tiles = [[] for _ in range(nhalf)]
    s_base = []
    sw_sb = []
    s_entries = []  # (sb, tile, hpar, lo_abs, w, scol)
    scol = 0
    for s in range(nsb):
        s_base.append(scol)
        for hp in range(hpb):
            h = s * hpb + hp
            if h >= nhalf:
                break
            for c in range(nchunk):
                t0, t1 = int(tile_start[s * nchunk + c]), int(tile_start[s * nchunk + c + 1])
                for t in range(t0, t1):
                    k = t * hpb + hp
                    if wmax[k] < 0:
                        continue
                    lo = int(wmin[k]) - hp * HW
                    w = int(wmax[k]) - hp * HW + 1 - lo
                    half_tiles[h].append((t - sb_base[s], lo, w, scol - s_base[s]))
                    s_entries.append((t, hp * HW + lo, w))
                    scol += w
            assert half_tiles[h], f"half {h} has no tiles"
        sw_sb.append(scol - s_base[s])
    stot = scol
    max_sw = max(sw_sb)

    s_all = np.zeros((NCORES, P, stot), dtype=bf16)
    sf = np.empty((P, 512), dtype=np.float32)
    for m in range(NCORES):
        dlm = dl_all[m]
        buf = np.zeros((P, stot), dtype=np.float32)
        for (t, lo_abs, w), sc in zip(s_entries, _scols(s_entries)):
            np.equal(
                dlm[t][:, None],
                np.arange(lo_abs, lo_abs + w, dtype=np.float32)[None, :],
                out=sf[:, 0:w],
            )
            buf[:, sc : sc + w] = sf[:, 0:w]
        s_all[m] = buf.astype(bf16)

    # ---- small per-core arrays (permuted node order) ----------------------
    ccrow = np.zeros((NCORES, 1, npc_pad), dtype=bf16)
    dsb = np.ones((NCORES, P, nblk), dtype=np.float32)
    pos = np.arange(npc, dtype=np.int64)
    for m in range(NCORES):
        nodes = perm[pos * NCORES + m]
        ccrow[m, 0, :npc] = cc[nodes].astype(np.float32)
        dm = np.ones(npc_pad, dtype=np.float64)
        dm[:npc] = d[nodes]
        dsb[m] = dm.reshape(nblk, P).T.astype(np.float32)

    meta = dict(
        N=N, din=din, dout=dout, npc=npc, nblk=nblk, npc_pad=npc_pad,
        nchunk=nchunk, chunk=chunk, n_y=n_y, nsb=nsb, nhalf=nhalf, ttot=ttot,
        stot=stot, sb_calls=sb_calls, sb_base=sb_base, s_base=s_base,
        sw_sb=sw_sb, max_sb_tiles=max_sb_tiles, sb_tiles=sb_tiles, max_sw=max_sw,
        half_tiles=half_tiles,
    )
    data = dict(y=y, idx_all=idx_all, s_all=s_all, ccrow=ccrow, dsb=dsb, perm=perm)
    return meta, data


def _scols(s_entries):
    sc = 0
    for (_, _, w) in s_entries:
        yield sc
        sc += w


def _meta_key(meta):
    return (
        meta["N"], meta["din"], meta["dout"], meta["ttot"], meta["stot"],
        meta["max_sb_tiles"], meta["max_sw"],
        tuple(tuple(t) for tl in meta["half_tiles"] for t in tl),
        tuple(tuple(c) for cl in meta["sb_calls"] for c in cl),
    )


def kernel(x, edge_index, W, b):
    x = np.asarray(x, dtype=np.float32)
    W = np.asarray(W, dtype=np.float32)
    b = np.asarray(b, dtype=np.float32)
    edge_index = np.asarray(edge_index)
    meta, data = _prep(x, edge_index, W, b)
    N, din, dout = meta["N"], meta["din"], meta["dout"]

    key = ("l", _meta_key(meta))
    if key not in _cache:
        _cache[key] = _build(meta)
    nc = _cache[key]

    wt = np.ascontiguousarray(W.T)
    import ml_dtypes
    brow = b[None, :].astype(np.dtype(ml_dtypes.bfloat16))
    in_maps = [
        {
            "y_t": data["y"],
            "idx_t": data["idx_all"][m],
            "s_t": data["s_all"][m],
            "wt_t": wt,
            "brow_t": brow,
            "ccrow_t": data["ccrow"][m],
            "dsb_t": data["dsb"][m],
        }
        for m in range(NCORES)
    ]
    res = run_bass_kernel_spmd(nc, in_maps, list(range(NCORES))).results

    LAST.clear()
    LAST.update(nc=nc, in_maps=in_maps, meta=meta)

    out = np.empty((N, dout), dtype=np.float32)
    perm = data["perm"]
    pos = np.arange(meta["npc"], dtype=np.int64)
    for m in range(NCORES):
        out[perm[pos * NCORES + m]] = res[m]["out_t"][: meta["npc"]]
    return out


# revision 17
# speedup vs baseline: 326.9430x; 1.0490x over previous
"""GCN layer kernel for Trainium2, 8 NeuronCores — single launch.

Math (identical to reference):
    deg = bincount(row);  d = 1/sqrt(deg)
    h   = x @ W.T + b
    out = d * segment_sum(d[col] * h[col], row) + d^2 * h

Aggregate-then-transform (linear map commutes with the segment sum):
    y[j]   = d_j * x_j                      (host, bf16)
    U[r]   = sum_{edges (r,c)} y[c] + y[r]  (self term = extra edge slot)
    cc[r]  = sum_{edges (r,c)} d_c + d_r    (host)
    out[r] = d_r * (U[r] @ W.T + cc[r] * b)

Device program (SPMD over 8 cores, destinations sharded):
  * destinations are dealt to (core, position) round-robin in descending
    degree order, which equalizes per-core work per gather call and so
    minimizes the cross-core-max padding the static SPMD schedule needs.
  * edges sorted by (dest superblock of SBD, source chunk, dest); bulk
    gathered with gpsimd.dma_gather (256B bf16 y rows) spread round-robin
    over 4 SWDGE queues (a single queue runs at only ~27 GB/s).  Gathered
    edge i lands at SBUF partition i%128, tile i//128.
  * per 128-edge tile a 0/1 selection matrix S[edge, dest_local] over the
    tile's dest window is PRECOMPUTED ON HOST (bf16) and streamed in via
    regular DMA (~20 MB/core; cheaper than building it on DVE, which costs
    ~290 ns/instruction on HW).  One PE matmul per tile accumulates
    G^T @ S into the 512-dest half's PSUM bank as U^T [feat, dest] — the
    final W matmul reads that as lhsT directly, so there are no transposes.
  * per 128-dest stripe: o2 = U_T_stripe^T @ W^T plus a rank-1 cc x b
    matmul into the same PSUM, then one activation copy scaled by d -> out.
"""

import numpy as np
import sys

sys.path.insert(0, "/opt/trn_rl_repo")

import concourse.bacc as bacc
import concourse.tile as tile
from concourse import mybir
from concourse.bass_utils import run_bass_kernel_spmd

NCORES = 8
P = 128
MAXCHUNK = 32000  # dma_gather idx is int16: source chunks must stay < 32768 rows
SBD = 1024  # dests per superblock (gather-slab granularity)
HW = 512  # dests per PSUM half (one 2KB fp32 bank)
NQUEUES = 4
F32 = mybir.dt.float32
I16 = mybir.dt.int16
BF16 = mybir.dt.bfloat16
FP8 = mybir.dt.float8e4

_cache = {}
LAST = {}  # populated on each kernel() call (for profiling in test.py)


def _build(meta, nrep=1, mode="full"):
    din = meta["din"]
    dout = meta["dout"]
    chunk = meta["chunk"]
    n_y = meta["n_y"]
    npc_pad = meta["npc_pad"]
    nblk = meta["nblk"]  # 128-dest stripes per core
    nsb = meta["nsb"]
    nhalf = meta["nhalf"]
    ttot = meta["ttot"]
    stot = meta["stot"]
    sb_calls = meta["sb_calls"]  # per sb: list of (chunk, tile_off_in_sb, ntiles)
    sb_base = meta["sb_base"]  # per sb: global tile offset
    s_base = meta["s_base"]  # per sb: S column offset
    sw_sb = meta["sw_sb"]  # per sb: S columns
    max_sb_tiles = meta["max_sb_tiles"]
    sb_tiles = meta["sb_tiles"]
    max_sw = meta["max_sw"]
    half_tiles = meta["half_tiles"]  # per half: list of (tile_in_sb, lo, w, soff_in_sb)

    nc = bacc.Bacc(
        "TRN2",
        target_bir_lowering=False,
        debug=False,
        enable_asserts=False,
        num_devices=NCORES,
        num_swdge_queues=NQUEUES,
    )
    y_t = nc.dram_tensor("y_t", [n_y, din], BF16, kind="ExternalInput").ap()
    idx_t = nc.dram_tensor("idx_t", [P, ttot * 8], I16, kind="ExternalInput").ap()
    s_t = nc.dram_tensor("s_t", [P, stot], FP8, kind="ExternalInput").ap()
    wt_t = nc.dram_tensor("wt_t", [din, dout], F32, kind="ExternalInput").ap()
    brow_t = nc.dram_tensor("brow_t", [1, dout], BF16, kind="ExternalInput").ap()
    ccrow_t = nc.dram_tensor("ccrow_t", [1, npc_pad], BF16, kind="ExternalInput").ap()
    dsb_t = nc.dram_tensor("dsb_t", [P, nblk], F32, kind="ExternalInput").ap()
    out_t = nc.dram_tensor("out_t", [npc_pad, dout], F32, kind="ExternalOutput").ap()
    out_v = out_t.rearrange("(t p) f -> p t f", p=P)

    with tile.TileContext(nc) as tc:
        with (
            tc.tile_pool(name="const", bufs=1) as cpool,
            tc.tile_pool(name="slab", bufs=2) as gpool,
            tc.tile_pool(name="sslab", bufs=2) as spool,
            tc.tile_pool(name="idxp", bufs=2) as ipool,
            tc.tile_pool(name="work", bufs=3) as wpool,
            tc.tile_pool(name="out", bufs=2) as opool,
            tc.tile_pool(name="psU", bufs=3, space="PSUM") as ppool,
            tc.tile_pool(name="psO", bufs=2, space="PSUM") as p2pool,
        ):
            wt_sb = cpool.tile([din, dout], dtype=F32)
            nc.sync.dma_start(out=wt_sb[:], in_=wt_t[:, :])
            brow_sb = cpool.tile([1, dout], dtype=BF16)
            nc.sync.dma_start(out=brow_sb[:], in_=brow_t[:, :])
            ccrow_sb = cpool.tile([1, npc_pad], dtype=BF16)
            nc.sync.dma_start(out=ccrow_sb[:], in_=ccrow_t[:, :])
            dsb_sb = cpool.tile([P, nblk], dtype=F32)
            nc.sync.dma_start(out=dsb_sb[:], in_=dsb_t[:, :])
            zs = cpool.tile([P, HW], dtype=FP8)
            nc.gpsimd.memset(zs[:], 0.0)

            def body():
                qrr = [0]
                for sb in range(nsb):
                    tb = sb_base[sb]
                    nt_sb = sb_tiles[sb]
                    idx_sb = ipool.tile([P, max_sb_tiles * 8], dtype=I16, tag="idx")
                    nc.sync.dma_start(
                        out=idx_sb[:, 0 : nt_sb * 8],
                        in_=idx_t[:, tb * 8 : (tb + nt_sb) * 8],
                    )
                    slab = gpool.tile([P, max_sb_tiles, din], dtype=BF16, tag="slab")
                    for (c, toff, nt) in sb_calls[sb] if mode != "nogather" else []:
                        c_lo = c * chunk
                        c_hi = min((c + 1) * chunk, n_y)
                        nc.gpsimd.dma_gather(
                            out_ap=slab[:, toff : toff + nt, :],
                            in_ap=y_t[c_lo:c_hi, :],
                            idxs_ap=idx_sb[:, toff * 8 : (toff + nt) * 8],
                            num_idxs=nt * P,
                            num_idxs_reg=nt * P,
                            elem_size=din,
                            single_packet=False,
                            queue_num=qrr[0] % NQUEUES,
                        )
                        qrr[0] += 1
                    sslab = spool.tile([P, max_sw], dtype=FP8, tag="sslab")
                    nc.scalar.dma_start(
                        out=sslab[:, 0 : sw_sb[sb]],
                        in_=s_t[:, s_base[sb] : s_base[sb] + sw_sb[sb]],
                    )
                    for hpar in range(SBD // HW):
                        h = sb * (SBD // HW) + hpar
                        if h >= nhalf:
                            break
                        if mode == "gather":
                            nq = min(HW // P, nblk - h * (HW // P))
                            ob = opool.tile([P, HW // P, dout], dtype=F32, tag="ob")
                            for q in range(nq):
                                nc.scalar.activation(
                                    ob[:, q, :], wt_sb[:],
                                    mybir.ActivationFunctionType.Copy,
                                )
                            nc.sync.dma_start(
                                out=out_v[:, h * (HW // P) : h * (HW // P) + nq, :],
                                in_=ob[:, 0:nq, :],
                            )
                            continue
                        tiles = half_tiles[h]
                        cw = min(HW, npc_pad - h * HW)
                        U = ppool.tile([P, HW], dtype=F32, space="PSUM", tag="U")
                        ntb = len(tiles)
                        nc.tensor.matmul(
                            out=U[:, 0:cw],
                            lhsT=slab[:, 0, :],
                            rhs=zs[:, 0:cw],
                            start=True,
                            stop=False,
                        )
                        for ti, (tloc, lo, w, soff) in enumerate(tiles):
                            nc.tensor.matmul(
                                out=U[:, lo : lo + w],
                                lhsT=slab[:, tloc, :],
                                rhs=sslab[:, soff : soff + w],
                                start=False,
                                stop=(ti == ntb - 1),
                            )
                        ut = wpool.tile([P, HW], dtype=F32, tag="ut")
                        nc.scalar.activation(
                            ut[:], U[:], mybir.ActivationFunctionType.Copy
                        )
                        nq = min(HW // P, nblk - h * (HW // P))
                        ob = opool.tile([P, HW // P, dout], dtype=F32, tag="ob")
                        for q in range(nq):
                            g = h * (HW // P) + q
                            o2 = p2pool.tile([P, dout], dtype=F32, space="PSUM", tag="o2")
                            nc.tensor.matmul(
                                out=o2[:],
                                lhsT=ut[:, q * P : (q + 1) * P],
                                rhs=wt_sb[:],
                                start=True,
                                stop=False,
                            )
                            nc.tensor.matmul(
                                out=o2[:],
                                lhsT=ccrow_sb[0:1, g * P : (g + 1) * P],
                                rhs=brow_sb[0:1, :],
                                start=False,
                                stop=True,
                            )
                            nc.scalar.activation(
                                ob[:, q, :],
                                o2[:],
                                mybir.ActivationFunctionType.Copy,
                                scale=dsb_sb[:, g : g + 1],
                            )
                        nc.sync.dma_start(
                            out=out_v[:, h * (HW // P) : h * (HW // P) + nq, :],
                            in_=ob[:, 0:nq, :],
                        )

            if nrep > 1:
                with tc.For_i(0, nrep, 1):
                    body()
            else:
                body()
    nc.compile()
    return nc


def _prep(x, edge_index, W, b):
    import ml_dtypes

    bf16 = np.dtype(ml_dtypes.bfloat16)
    N, din = x.shape
    dout = W.shape[0]
    npc = N // NCORES
    assert npc * NCORES == N
    nblk = (npc + P - 1) // P
    npc_pad = nblk * P
    nhalf = (npc_pad + HW - 1) // HW
    hpb = SBD // HW
    nsb = (nhalf + hpb - 1) // hpb
    nchunk = (N + MAXCHUNK - 1) // MAXCHUNK
    chunk = (N + nchunk - 1) // nchunk
    n_y = N

    row = np.asarray(edge_index[0], dtype=np.int64)
    col = np.asarray(edge_index[1], dtype=np.int64)
    deg = np.bincount(row, minlength=N)
    d = 1.0 / np.sqrt(deg.astype(np.float64))
    y = (x.astype(np.float64) * d[:, None]).astype(np.float32).astype(bf16)
    cc = d + np.bincount(row, weights=d[col], minlength=N)

    # destination -> (core, position): deal in descending-degree order so all
    # cores see near-identical per-call work (minimizes cross-core-max pad).
    perm = np.argsort(-deg, kind="stable")  # perm[rank] = node
    rank = np.empty(N, dtype=np.int64)
    rank[perm] = np.arange(N)

    # ---- slots = edges + self edges, dest-sharded -------------------------
    rows_a = np.concatenate([row, np.arange(N, dtype=np.int64)])
    cols_a = np.concatenate([col, np.arange(N, dtype=np.int64)])
    r = rank[rows_a]
    core = r % NCORES
    rl = r // NCORES  # dest position within core, 0..npc-1
    sb = rl // SBD
    ch = cols_a // chunk
    sbg = core * nsb + sb
    order = np.lexsort((rl, ch, sbg))
    core_s, rl_s, ch_s, cols_s = core[order], rl[order], ch[order], cols_a[order]
    sb_s = rl_s // SBD
    gid = (core_s * nsb + sb_s) * nchunk + ch_s
    ngrp = NCORES * nsb * nchunk
    gcnt = np.bincount(gid, minlength=ngrp).reshape(NCORES, nsb * nchunk)
    tcnt = (gcnt.max(axis=0) + P - 1) // P  # [nsb*nchunk]
    tile_start = np.zeros(nsb * nchunk + 1, dtype=np.int64)
    np.cumsum(tcnt, out=tile_start[1:])
    ttot = int(tile_start[-1])
    sb_base = [int(tile_start[s * nchunk]) for s in range(nsb)]
    sb_tiles = [
        int(tile_start[(s + 1) * nchunk] - tile_start[s * nchunk]) for s in range(nsb)
    ]
    max_sb_tiles = max(sb_tiles)

    sb_calls = []
    for s in range(nsb):
        calls = []
        for c in range(nchunk):
            nt = int(tcnt[s * nchunk + c])
            if nt:
                calls.append((c, int(tile_start[s * nchunk + c]) - sb_base[s], nt))
        sb_calls.append(calls)

    # ---- per-core slot data ----------------------------------------------
    grp_start = np.zeros(ngrp + 1, dtype=np.int64)
    np.cumsum(np.bincount(gid, minlength=ngrp), out=grp_start[1:])
    rank_in_g = np.arange(len(gid), dtype=np.int64) - grp_start[gid]
    gnc = gid % (nsb * nchunk)
    slot = tile_start[gnc] * P + rank_in_g
    tno = slot // P
    pno = slot - tno * P
    lidx = (cols_s - ch_s * chunk).astype(np.int16)
    dl10 = rl_s - sb_s * SBD  # 0..SBD-1 within superblock

    idx_all = np.zeros((NCORES, P, ttot * 8), dtype=np.int16)
    dl_all = np.full((NCORES, ttot, P), -1.0, dtype=np.float32)
    nkey = ttot * hpb
    wmin = np.full(nkey, SBD, dtype=np.int64)
    wmax = np.full(nkey, -1, dtype=np.int64)
    for m in range(NCORES):
        sel = core_s == m
        flat = np.zeros((ttot, P), dtype=np.int16)
        flat[tno[sel], pno[sel]] = lidx[sel]
        wrapped = flat.reshape(ttot, 8, 16).transpose(2, 0, 1).reshape(16, ttot * 8)
        idx_all[m] = np.tile(wrapped, (8, 1))
        dl_all[m][tno[sel], pno[sel]] = dl10[sel].astype(np.float32)
        key = tno[sel] * hpb + dl10[sel] // HW
        np.minimum.at(wmin, key, dl10[sel])
        np.maximum.at(wmax, key, dl10[sel])

    # ---- per-half tile schedule + packed host-built S ---------------------
    half_tiles = [[] for _ in range(nhalf)]
    s_base = []
    sw_sb = []
    s_entries = []  # (sb, tile, hpar, lo_abs, w, scol)
    scol = 0
    for s in range(nsb):
        s_base.append(scol)
        for hp in range(hpb):
            h = s * hpb + hp
            if h >= nhalf:
                break
            for c in range(nchunk):
                t0, t1 = int(tile_start[s * nchunk + c]), int(tile_start[s * nchunk + c + 1])
                for t in range(t0, t1):
                    k = t * hpb + hp
                    if wmax[k] < 0:
                        continue
                    lo = int(wmin[k]) - hp * HW
                    w = int(wmax[k]) - hp * HW + 1 - lo
                    half_tiles[h].append((t - sb_base[s], lo, w, scol - s_base[s]))
                    s_entries.append((t, hp * HW + lo, w))
                    scol += w
            assert half_tiles[h], f"half {h} has no tiles"
        sw_sb.append(scol - s_base[s])
    stot = scol
    max_sw = max(sw_sb)

    f8 = np.dtype(ml_dtypes.float8_e4m3)
    s_all = np.zeros((NCORES, P, stot), dtype=f8)
    sf = np.empty((P, 512), dtype=np.float32)
    for m in range(NCORES):
        dlm = dl_all[m]
        buf = np.zeros((P, stot), dtype=np.float32)
        for (t, lo_abs, w), sc in zip(s_entries, _scols(s_entries)):
            np.equal(
                dlm[t][:, None],
                np.arange(lo_abs, lo_abs + w, dtype=np.float32)[None, :],
                out=sf[:, 0:w],
            )
            buf[:, sc : sc + w] = sf[:, 0:w]
        s_all[m] = buf.astype(f8)

    # ---- small per-core arrays (permuted node order) ----------------------
    ccrow = np.zeros((NCORES, 1, npc_pad), dtype=bf16)
    dsb = np.ones((NCORES, P, nblk), dtype=np.float32)
    pos = np.arange(npc, dtype=np.int64)
    for m in range(NCORES):
        nodes = perm[pos * NCORES + m]
        ccrow[m, 0, :npc] = cc[nodes].astype(np.float32)
        dm = np.ones(npc_pad, dtype=np.float64)
        dm[:npc] = d[nodes]
        dsb[m] = dm.reshape(nblk, P).T.astype(np.float32)

    meta = dict(
        N=N, din=din, dout=dout, npc=npc, nblk=nblk, npc_pad=npc_pad,
        nchunk=nchunk, chunk=chunk, n_y=n_y, nsb=nsb, nhalf=nhalf, ttot=ttot,
        stot=stot, sb_calls=sb_calls, sb_base=sb_base, s_base=s_base,
        sw_sb=sw_sb, max_sb_tiles=max_sb_tiles, sb_tiles=sb_tiles, max_sw=max_sw,
        half_tiles=half_tiles,
    )
    data = dict(y=y, idx_all=idx_all, s_all=s_all, ccrow=ccrow, dsb=dsb, perm=perm)
    return meta, data


def _scols(s_entries):
    sc = 0
    for (_, _, w) in s_entries:
        yield sc
        sc += w


def _meta_key(meta):
    return (
        meta["N"], meta["din"], meta["dout"], meta["ttot"], meta["stot"],
        meta["max_sb_tiles"], meta["max_sw"],
        tuple(tuple(t) for tl in meta["half_tiles"] for t in tl),
        tuple(tuple(c) for cl in meta["sb_calls"] for c in cl),
    )


def kernel(x, edge_index, W, b):
    x = np.asarray(x, dtype=np.float32)
    W = np.asarray(W, dtype=np.float32)
    b = np.asarray(b, dtype=np.float32)
    edge_index = np.asarray(edge_index)
    meta, data = _prep(x, edge_index, W, b)
    N, din, dout = meta["N"], meta["din"], meta["dout"]

    key = ("l", _meta_key(meta))
    if key not in _cache:
        _cache[key] = _build(meta)
    nc = _cache[key]

    wt = np.ascontiguousarray(W.T)
    import ml_dtypes
    brow = b[None, :].astype(np.dtype(ml_dtypes.bfloat16))
    in_maps = [
        {
            "y_t": data["y"],
            "idx_t": data["idx_all"][m],
            "s_t": data["s_all"][m],
            "wt_t": wt,
            "brow_t": brow,
            "ccrow_t": data["ccrow"][m],
            "dsb_t": data["dsb"][m],
        }
        for m in range(NCORES)
    ]
    res = run_bass_kernel_spmd(nc, in_maps, list(range(NCORES))).results

    LAST.clear()
    LAST.update(nc=nc, in_maps=in_maps, meta=meta)

    out = np.empty((N, dout), dtype=np.float32)
    perm = data["perm"]
    pos = np.arange(meta["npc"], dtype=np.int64)
    for m in range(NCORES):
        out[perm[pos * NCORES + m]] = res[m]["out_t"][: meta["npc"]]
    return out
